# revision 1
# baseline (speedup 1.0000x reference)
"""Trainium2 Bass kernel for the TGM (temporal gradient matching) loss.

Strategy
--------
View pred/y/mask as [128 frames, L=518*518] matrices (B*N = 128 frames
exactly matches the PE contraction dim).  Shard the L (pixel) axis across
the 8 NeuronCores -- pairs couple adjacent *frames*, never pixels, so the
column shards are fully independent and need no halo.

Per core, stream column tiles [128, 1024] and compute all 124 in-batch
frame-pair differences at once on the TensorEngine:

    dG   = D^T  @ g      (D: +-1 bidiagonal "pair difference" matrix, f32)
    dG  += W2^T @ m      (W2 = -D * diag(rc) * 512 folds the valid-mask in:
                          the "poison" trick.  rc[f] = 64*(1+f), so any pair
                          with a masked-out endpoint lands >= ~64 away from
                          the in-range value; rc gaps are the constant 64,
                          which the ScalarE bias adds back.  The x512
                          compensates the fp8 reinterpretation of the mask
                          bytes: 0x01 as float8_e4m3 is 2^-9.)
    dP   = Dbf^T @ p     (bf16 -- only perturbs the value path ~1e-4 rel)

    adg  = |dG + 64|            (ScalarE Abs, per-partition bias)
    adp  = |dP|                 (ScalarE Abs)
    st0  = (adg < 0.05)         (DVE tensor_scalar, fused accum -> num)
    dd   = adp - adg            (DVE)
    dm   = dd * st0             (DVE; st0>=0 so |dm| == |dd|*st0)
    sum += |dm|                 (ScalarE Abs with fused accum_out)

DMA: two parallel rings (all DGE queues share one AXI port; this split
measured fastest, ~250 GB/s combined reads) -- p cast f32->bf16 plus the
fp8-viewed masks on the SWDGE ring, g f32 on the qSP HWDGE queue.

The per-pair num/sum partials accumulate into [124, ngroups] SBUF buffers,
reduced at the end and DMA'd out; the host sums across cores and applies
the final ratio/mean in float64.
"""

import os
import sys

import numpy as np

sys.path.insert(0, "/opt/trn_rl_repo")

import concourse.bacc as bacc  # noqa: E402
import concourse.bass as bass  # noqa: E402
import concourse.tile as tile  # noqa: E402
from concourse import bass_utils, mybir  # noqa: E402

# Problem geometry (hardcoded per contest rules).
B, N, H, W = 4, 32, 518, 518
NF = B * N              # 128 frames
NPAIR = B * (N - 1)     # 124 in-batch adjacent pairs
L = H * W               # 268324 pixels per frame
NCORES = 8

GRP = 1024              # columns per compute group (2 PSUM banks)
MM_F = 512              # matmul moving free dim (1 PSUM bank)
NGRP = 33               # groups per core
C = GRP * NGRP          # 33792 columns per core
LPAD = C * NCORES       # 270336 padded pixel count

BIG = 64.0              # poison magnitude / ScalarE bias
STATIC_THRESH = 0.05

_f32 = mybir.dt.float32
_bf16 = mybir.dt.bfloat16
_fp8 = mybir.dt.float8e4
FP8_ONE_INV = 512.0  # 1 / float8_e4m3(0x01); mask bytes reinterpret as fp8
_ALU = mybir.AluOpType
_ACTF = mybir.ActivationFunctionType

_COMPILED = None
_LAST_RESULTS = None


def make_weights():
    """D (pair difference) and W2 (mask poison) stationary matrices."""
    d_w = np.zeros((NF, NPAIR), dtype=np.float32)
    w2_w = np.zeros((NF, NPAIR), dtype=np.float32)
    rc = BIG * (1.0 + np.arange(NF, dtype=np.float32))
    p = 0
    for b in range(B):
        for i in range(N - 1):
            f = b * N + i
            d_w[f, p] = -1.0
            d_w[f + 1, p] = 1.0
            # PSUM accumulation adds, so W2 carries the minus sign:
            # psum = D^T g + W2^T m = dG - rc_c*m_c + rc_f*m_f = dG - BIG
            # (valid case).  rc*512*(1+f) stays bf16-exact.
            w2_w[f, p] = rc[f] * FP8_ONE_INV
            w2_w[f + 1, p] = -rc[f + 1] * FP8_ONE_INV
            p += 1
    return d_w, w2_w


def build_program(cols_per_core=C, grp=GRP):
    ngrp = cols_per_core // grp
    assert ngrp * grp == cols_per_core
    nc = bacc.Bacc(
        "TRN2", target_bir_lowering=False, debug=False, num_devices=NCORES
    )
    p_in = nc.dram_tensor("p_in", [NF, cols_per_core], _f32, kind="ExternalInput").ap()
    g_in = nc.dram_tensor("g_in", [NF, cols_per_core], _f32, kind="ExternalInput").ap()
    m_in = nc.dram_tensor("m_in", [NF, cols_per_core], _fp8, kind="ExternalInput").ap()
    dw_in = nc.dram_tensor("d_w", [NF, NPAIR], _f32, kind="ExternalInput").ap()
    dbf_in = nc.dram_tensor("d_bf", [NF, NPAIR], _bf16, kind="ExternalInput").ap()
    w2bf_in = nc.dram_tensor("w2_bf", [NF, NPAIR], _bf16, kind="ExternalInput").ap()
    num_out = nc.dram_tensor("num_out", [NPAIR, 1], _f32, kind="ExternalOutput").ap()
    sum_out = nc.dram_tensor("sum_out", [NPAIR, 1], _f32, kind="ExternalOutput").ap()

    with tile.TileContext(nc) as tc:
        with (
            tc.tile_pool(name="consts", bufs=1) as cpool,
            tc.tile_pool(name="io", bufs=6) as iopool,
            tc.tile_pool(name="mid", bufs=3) as midpool,
            tc.tile_pool(name="acc", bufs=1) as accpool,
            tc.tile_pool(name="psum", bufs=2, space="PSUM") as pspool,
        ):
            d_sb = cpool.tile([NF, NPAIR], _f32, name="d_sb")
            dbf_sb = cpool.tile([NF, NPAIR], _bf16, name="dbf_sb")
            w2bf_sb = cpool.tile([NF, NPAIR], _bf16, name="w2bf_sb")
            # Weight tables ride the otherwise-idle qAct queue so the first
            # g-tile isn't queued behind them on the qSP ring.
            nc.scalar.dma_start(out=d_sb[:], in_=dw_in[:])
            nc.scalar.dma_start(out=dbf_sb[:], in_=dbf_in[:])
            nc.scalar.dma_start(out=w2bf_sb[:], in_=w2bf_in[:])
            bias_sb = cpool.tile([NPAIR, 1], _f32, name="bias_sb")
            nc.vector.memset(bias_sb[:], BIG)
            zero_sb = cpool.tile([NPAIR, 1], _f32, name="zero_sb")
            nc.vector.memset(zero_sb[:], 0.0)
            num_buf = accpool.tile([NPAIR, ngrp], _f32, name="num_buf")
            sum_buf = accpool.tile([NPAIR, ngrp], _f32, name="sum_buf")

            for t in range(ngrp):
                sl = bass.ts(t, grp)
                # Two parallel DMA rings (queues share the AXI port; this
                # mix measured fastest): p cast f32->bf16 and m (as fp8) on
                # the SWDGE ring, g f32 on qSP HWDGE at 4KB rows.
                pt = iopool.tile([NF, grp], _bf16, tag="pt", name=f"pt{t}")
                gt = iopool.tile([NF, grp], _f32, tag="gt", name=f"gt{t}")
                mt = iopool.tile([NF, grp], _fp8, tag="mt", name=f"mt{t}")
                # m first in the SWDGE ring FIFO: the small mask tile lands
                # before the big p transfer, so the G-side matmuls can start
                # as soon as g arrives on the other ring.
                nc.gpsimd.dma_start(out=mt[:], in_=m_in[:, sl])
                nc.gpsimd.dma_start(out=pt[:], in_=p_in[:, sl])
                nc.sync.dma_start(out=gt[:], in_=g_in[:, sl])

                ps_g = pspool.tile([NPAIR, grp], _f32, tag="ps_g", name=f"psg{t}")
                ps_p = pspool.tile([NPAIR, grp], _f32, tag="ps_p", name=f"psp{t}")
                for h in range(grp // MM_F):
                    hs = bass.ts(h, MM_F)
                    nc.tensor.matmul(
                        ps_g[:, hs], d_sb[:], gt[:, hs], start=True, stop=False
                    )
                    nc.tensor.matmul(
                        ps_g[:, hs], w2bf_sb[:], mt[:, hs], start=False, stop=True
                    )
                    nc.tensor.matmul(
                        ps_p[:, hs], dbf_sb[:], pt[:, hs], start=True, stop=True
                    )

                adg = midpool.tile([NPAIR, grp], _f32, tag="adg", name=f"adg{t}")
                adp = midpool.tile([NPAIR, grp], _f32, tag="adp", name=f"adp{t}")
                st0 = midpool.tile([NPAIR, grp], _f32, tag="st0", name=f"st0{t}")
                dd = midpool.tile([NPAIR, grp], _f32, tag="dd", name=f"dd{t}")
                dm = midpool.tile([NPAIR, grp], _f32, tag="dm", name=f"dm{t}")

                # adg = |psum_g + BIG|; in the valid case psum_g = dG - BIG.
                nc.scalar.activation(
                    adg[:], ps_g[:], _ACTF.Abs, bias=bias_sb[:], scale=1.0
                )
                nc.scalar.activation(
                    adp[:], ps_p[:], _ACTF.Abs, bias=zero_sb[:], scale=1.0
                )
                # st0 = (adg < thresh), fused accum -> num partial; op1
                # doubles as the accumulate-reduce op when accum_out is set.
                nc.vector.tensor_scalar(
                    st0[:],
                    adg[:],
                    STATIC_THRESH,
                    None,
                    _ALU.is_lt,
                    _ALU.add,
                    accum_out=num_buf[:, t : t + 1],
                )
                nc.vector.tensor_tensor(dd[:], adp[:], adg[:], _ALU.subtract)
                # dm = dd * st0 (signed, masked); ScalarE then computes |dm|
                # with the free accumulate -> sum partial.
                nc.vector.tensor_tensor(dm[:], dd[:], st0[:], _ALU.mult)
                ab = midpool.tile([NPAIR, grp], _f32, tag="ab", name=f"ab{t}")
                nc.scalar.activation(
                    ab[:],
                    dm[:],
                    _ACTF.Abs,
                    bias=zero_sb[:],
                    scale=1.0,
                    accum_out=sum_buf[:, t : t + 1],
                )

            nr = accpool.tile([NPAIR, 1], _f32, name="nr")
            sr = accpool.tile([NPAIR, 1], _f32, name="sr")
            nc.vector.tensor_reduce(
                nr[:], num_buf[:], mybir.AxisListType.X, _ALU.add
            )
            nc.vector.tensor_reduce(
                sr[:], sum_buf[:], mybir.AxisListType.X, _ALU.add
            )
            nc.sync.dma_start(out=num_out[:], in_=nr[:])
            nc.sync.dma_start(out=sum_out[:], in_=sr[:])

    nc.compile()
    return nc


def _get_compiled():
    global _COMPILED
    if _COMPILED is None:
        _COMPILED = build_program()
    return _COMPILED


def kernel(pred, y, masks_squeezed):
    global _LAST_RESULTS
    nc = _get_compiled()

    pred = np.asarray(pred, dtype=np.float32).reshape(NF, L)
    y = np.asarray(y, dtype=np.float32).reshape(NF, L)
    m = np.asarray(masks_squeezed).reshape(NF, L).view(np.uint8)

    import ml_dtypes

    d_w, w2_w = make_weights()
    d_bf = d_w.astype(ml_dtypes.bfloat16)
    w2_bf = w2_w.astype(ml_dtypes.bfloat16)
    # rc values (64*512*(1+f), f<128) are exactly representable in bf16
    assert np.array_equal(w2_bf.astype(np.float32), w2_w)

    def pad(a, dt):
        out = np.zeros((NF, LPAD), dtype=dt)
        out[:, :L] = a
        return out

    p_pad = pad(pred, np.float32)
    g_pad = pad(y, np.float32)
    m_pad = pad(m, np.uint8)

    in_maps = []
    for k in range(NCORES):
        sl = slice(k * C, (k + 1) * C)
        in_maps.append(
            {
                "p_in": np.ascontiguousarray(p_pad[:, sl]),
                "g_in": np.ascontiguousarray(g_pad[:, sl]),
                # bit-level reinterpretation: mask byte 0x01 == fp8e4m3 2^-9
                "m_in": np.ascontiguousarray(m_pad[:, sl]).view(
                    mybir.dt.np(_fp8)
                ),
                "d_w": d_w,
                "d_bf": d_bf,
                "w2_bf": w2_bf,
            }
        )

    res = bass_utils.run_bass_kernel_spmd(
        nc,
        in_maps,
        core_ids=list(range(NCORES)),
        trace=bool(int(os.environ.get("TGM_TRACE", "0"))),
    )
    _LAST_RESULTS = res

    num = np.zeros(NPAIR, dtype=np.float64)
    ssum = np.zeros(NPAIR, dtype=np.float64)
    for r in res.results:
        num += r["num_out"][:, 0].astype(np.float64)
        ssum += r["sum_out"][:, 0].astype(np.float64)

    tgm = np.where(num > 0, ssum / np.maximum(num, 1.0), 0.0)
    loss = tgm.sum() / float((N - 1) * B)
    return np.asarray(loss, dtype=np.float32)

